# revision 5
# baseline (speedup 1.0000x reference)
"""Trainium2 Bass kernel for nn_MoEFFNBlock (B=2,S=2048,D=1024,H=2048,E=8,K=2).

Strategy (expert-parallel, 8 cores):
  host: fp32 router (softmax+top2, validated to match the jax reference
        selection), gather tokens per expert, fold the normalized top-k
        combine weight into the expert output on-device.
  core e: expert-e SwiGLU FFN over its <=C gathered tokens with fp32r
        matmuls (full PE rate, ~1e-4 rel err), plus a 256-wide H-shard of
        the shared expert over all T tokens.
  host: scatter-add per-expert outputs + sum the 8 shared-expert partials.

All matmul operands are laid out on host so every DMA is a large
contiguous transfer: weights pre-tiled to [tile, 128, kouter, 128].
"""

import json
import math

import numpy as np

_B, _S, _D, _H, _E = 2, 2048, 1024, 2048, 8
_T = _B * _S
_P = 128
_NC = 8
_HSH = _H // _NC  # shared-expert H columns per core
_DK = _D // _P  # 8 contraction tiles over D
_HT = _H // _P  # 16 tiles over H
_SK = _HSH // _P  # 2 contraction tiles over the H-shard
_TC = 512  # shared-expert token chunk

_TPB_ENGINES = {"PE", "Activation", "DVE", "Pool", "SP"}


def _split_waits(bir_bytes: bytes) -> bytes:
    """walrus in this container accepts only one sync-wait per TPB
    instruction; Tile's tail drain carries several. Hoist extras onto
    NoOps that run just before the instruction on the same engine."""
    m = json.loads(bir_bytes)
    ctr = 0
    for f in m["functions"]:
        blocks = f["blocks"]
        items = blocks.items() if isinstance(blocks, dict) else enumerate(blocks)
        for _bname, bb in items:
            new_insts = []
            for inst in bb["instructions"]:
                si = inst.get("sync_info") or {}
                ow = si.get("on_wait") or []
                if len(ow) > 1 and inst.get("engine") in _TPB_ENGINES:
                    for w in ow[:-1]:
                        ctr += 1
                        nop = {
                            "name": f"I-waitsplit-{ctr}",
                            "engine": inst["engine"],
                            "opcode": "NoOp",
                            "ins": [],
                            "outs": [],
                            "sync_info": {"on_wait": [w], "on_update": []},
                        }
                        if "debug" in inst:
                            nop["debug"] = inst["debug"]
                        new_insts.append(nop)
                    si["on_wait"] = [ow[-1]]
                new_insts.append(inst)
            bb["instructions"] = new_insts
    return json.dumps(m).encode()


def _chunks(C):
    """Column chunks of width 256..512 (fp32r needs >=256 cols for full PE
    rate). C must be a multiple of 128, C >= 512."""
    widths = []
    rem = C
    while rem >= 768:
        widths.append(512)
        rem -= 512
    if rem == 512:
        widths.append(512)
    elif rem >= 512:
        widths.append(rem - 256)
        widths.append(256)
    elif rem:
        widths.append(rem)
    ccs, o = [], 0
    for w in widths:
        assert 256 <= w <= 512
        ccs.append((o, w))
        o += w
    assert o == C
    return ccs


def _build(C):
    import concourse.bass as bass
    import concourse.mybir as mybir
    import concourse.tile as tile

    f32 = mybir.dt.float32
    f32r = mybir.dt.float32r
    Silu = mybir.ActivationFunctionType.Silu
    mult = mybir.AluOpType.mult

    nc = bass.Bass(trn_type="TRN2")
    xe = nc.dram_tensor("xe", [_P, _DK, C], f32r, kind="ExternalInput")
    cw = nc.dram_tensor("cw", [_P, C], f32, kind="ExternalInput")
    wg = nc.dram_tensor("wg", [_HT, _P, _DK, _P], f32r, kind="ExternalInput")
    wu = nc.dram_tensor("wu", [_HT, _P, _DK, _P], f32r, kind="ExternalInput")
    wd = nc.dram_tensor("wd", [_DK, _P, _HT, _P], f32r, kind="ExternalInput")
    xt = nc.dram_tensor("xt", [_P, _DK, _T], f32r, kind="ExternalInput")
    sg = nc.dram_tensor("sg", [_P, _DK, _HSH], f32r, kind="ExternalInput")
    su = nc.dram_tensor("su", [_P, _DK, _HSH], f32r, kind="ExternalInput")
    sd = nc.dram_tensor("sd", [_P, _SK, _D], f32r, kind="ExternalInput")
    rout = nc.dram_tensor("rout", [_DK, _P, C], f32, kind="ExternalOutput")
    shout = nc.dram_tensor("shout", [_DK, _P, _T], f32, kind="ExternalOutput")

    ccs = _chunks(C)

    with tile.TileContext(nc) as tc:
        with (
            tc.tile_pool(name="tmp", bufs=2) as tmp,
            tc.tile_pool(name="ps", bufs=2, space="PSUM") as psp,
            tc.tile_pool(name="bigS", bufs=1) as bigS,
            tc.tile_pool(name="cwg", bufs=1) as cwg,
        ):
            # PE warmup: dummy matmuls so HAM un-throttles while the
            # initial DMAs are in flight.
            wtile32 = cwg.tile([_P, 512], f32, name="wtile32")
            nc.vector.memset(wtile32[:], 0.0)
            wtile = cwg.tile([_P, 512], f32r, name="wtile")
            nc.vector.tensor_copy(wtile[:], wtile32[:])
            wps = psp.tile([_P, 512], f32, tag="warm", name="wps", bufs=1)
            for i in range(60):
                nc.tensor.matmul(
                    wps[:], wtile[:, :_P], wtile[:], start=(i == 0), stop=(i == 59)
                )

            # Shared-expert weights up-front (resident, 24 KB/part).
            sg_sb = bigS.tile([_P, _DK, _HSH], f32r)
            nc.sync.dma_start(sg_sb[:], sg.ap())
            su_sb = bigS.tile([_P, _DK, _HSH], f32r)
            nc.sync.dma_start(su_sb[:], su.ap())
            sd_sb = bigS.tile([_P, _SK, _D], f32r, name="sd_sb")
            nc.sync.dma_start(sd_sb[:], sd.ap())

            cw_sb = cwg.tile([_P, C], f32, name="cw_sb")
            nc.sync.dma_start(cw_sb[:], cw.ap())
            g_sb = cwg.tile([_P, _HT, C], f32r, name="g_sb")

            # ---------- phase R / h-stage: g = silu(Wg x) * (Wu x) * cw --
            with (
                tc.tile_pool(name="poolXE", bufs=1) as poolXE,
                tc.tile_pool(name="strGU", bufs=2) as strGU,
            ):
                xe_sb = poolXE.tile([_P, _DK, C], f32r, name="xe_sb")
                for c0, cn in ccs:
                    nc.sync.dma_start(
                        xe_sb[:, :, c0 : c0 + cn], xe.ap()[:, :, c0 : c0 + cn]
                    )

                for ht in range(_HT):
                    wg_t = strGU.tile([_P, _DK, _P], f32r, tag="wg", name="wg_t")
                    nc.sync.dma_start(wg_t[:], wg.ap()[ht])
                    wu_t = strGU.tile([_P, _DK, _P], f32r, tag="wu", name="wu_t")
                    nc.sync.dma_start(wu_t[:], wu.ap()[ht])
                    for c0, cn in ccs:
                        h1 = psp.tile([_P, 512], f32, tag="h1", name="h1ps")[:, :cn]
                        for k in range(_DK):
                            nc.tensor.matmul(
                                h1,
                                wg_t[:, k],
                                xe_sb[:, k, c0 : c0 + cn],
                                start=(k == 0),
                                stop=(k == _DK - 1),
                            )
                        h2 = psp.tile([_P, 512], f32, tag="h2", name="h2ps")[:, :cn]
                        for k in range(_DK):
                            nc.tensor.matmul(
                                h2,
                                wu_t[:, k],
                                xe_sb[:, k, c0 : c0 + cn],
                                start=(k == 0),
                                stop=(k == _DK - 1),
                            )
                        sl = tmp.tile([_P, 512], f32, tag="sl", name="sl_sb")[
                            :, :cn
                        ]
                        nc.scalar.activation(sl, h1, Silu)
                        t2 = tmp.tile([_P, 512], f32, tag="t2", name="t2_sb")[
                            :, :cn
                        ]
                        nc.vector.tensor_tensor(t2, h2, cw_sb[:, c0 : c0 + cn], mult)
                        nc.vector.tensor_tensor(
                            g_sb[:, ht, c0 : c0 + cn], sl, t2, mult
                        )

            # ---------- phase R / d-stage + phase S ----------------------
            with (
                tc.tile_pool(name="strDW", bufs=2) as strDW,
                tc.tile_pool(name="strS", bufs=3) as strS,
            ):
                for dt_i in range(_DK):
                    wd_t = strDW.tile([_P, _HT, _P], f32r, tag="wd", name="wd_t")
                    nc.sync.dma_start(wd_t[:], wd.ap()[dt_i])
                    for c0, cn in ccs:
                        ops = psp.tile([_P, 512], f32, tag="out", name="ops")[
                            :, :cn
                        ]
                        for k in range(_HT):
                            nc.tensor.matmul(
                                ops,
                                wd_t[:, k],
                                g_sb[:, k, c0 : c0 + cn],
                                start=(k == 0),
                                stop=(k == _HT - 1),
                            )
                        ro = tmp.tile([_P, 512], f32, tag="ro", name="ro_sb")[
                            :, :cn
                        ]
                        nc.vector.tensor_copy(ro, ops)
                        nc.sync.dma_start(rout.ap()[dt_i][:, c0 : c0 + cn], ro)

                # phase S, software-pipelined: d-stage trails one chunk.
                def s_hstage(t0):
                    xt_sb = strS.tile([_P, _DK, _TC], f32r, tag="xt", name="xt_sb")
                    nc.sync.dma_start(xt_sb[:], xt.ap()[:, :, t0 : t0 + _TC])
                    gs = strS.tile([_P, _SK, _TC], f32r, tag="gs", name="gs_sb")
                    for hs in range(_SK):
                        h1 = psp.tile([_P, 512], f32, tag="h1", name="h1ps")
                        for k in range(_DK):
                            nc.tensor.matmul(
                                h1,
                                sg_sb[:, k, hs * _P : (hs + 1) * _P],
                                xt_sb[:, k],
                                start=(k == 0),
                                stop=(k == _DK - 1),
                            )
                        h2 = psp.tile([_P, 512], f32, tag="h2", name="h2ps")
                        for k in range(_DK):
                            nc.tensor.matmul(
                                h2,
                                su_sb[:, k, hs * _P : (hs + 1) * _P],
                                xt_sb[:, k],
                                start=(k == 0),
                                stop=(k == _DK - 1),
                            )
                        sl = tmp.tile([_P, 512], f32, tag="sl", name="sl_sb")
                        nc.scalar.activation(sl, h1, Silu)
                        nc.vector.tensor_tensor(gs[:, hs], sl, h2, mult)
                    return gs

                def s_dstage(t0, gs):
                    for dt_i in range(_DK):
                        ops = psp.tile([_P, 512], f32, tag="out", name="ops")
                        for k in range(_SK):
                            nc.tensor.matmul(
                                ops,
                                sd_sb[:, k, dt_i * _P : (dt_i + 1) * _P],
                                gs[:, k],
                                start=(k == 0),
                                stop=(k == _SK - 1),
                            )
                        so = tmp.tile([_P, 512], f32, tag="ro", name="ro_sb")
                        nc.vector.tensor_copy(so, ops)
                        nc.sync.dma_start(shout.ap()[dt_i][:, t0 : t0 + _TC], so)

                prev = None
                for t0 in range(0, _T, _TC):
                    gs = s_hstage(t0)
                    if prev is not None:
                        s_dstage(prev[0], prev[1])
                    prev = (t0, gs)
                s_dstage(prev[0], prev[1])

    orig = nc.to_json_bytes
    nc.to_json_bytes = lambda: _split_waits(orig())
    return nc


def _route(xf, w_router):
    """fp32 router matching the jax reference: softmax over logits, top-2
    (selection identical to jax.lax.top_k for non-tied logits), weights
    renormalized over the selected pair."""
    logits = xf @ w_router.T.astype(np.float32)
    m = logits.max(-1, keepdims=True)
    p = np.exp(logits - m)
    p /= p.sum(-1, keepdims=True)
    i1 = p.argmax(-1)
    p2 = p.copy()
    p2[np.arange(p.shape[0]), i1] = -1.0
    i2 = p2.argmax(-1)
    w1 = p[np.arange(p.shape[0]), i1]
    w2 = p[np.arange(p.shape[0]), i2]
    s = w1 + w2
    return i1, i2, (w1 / s).astype(np.float32), (w2 / s).astype(np.float32)


def _tile_kxm(a2d, kouter):
    """[K, M] -> [128, K//128, M] with partition dim first."""
    k, mdim = a2d.shape
    assert k == kouter * _P
    return np.ascontiguousarray(a2d.reshape(kouter, _P, mdim).transpose(1, 0, 2))


def _prepare(inputs):
    x = np.asarray(inputs["x"], dtype=np.float32)
    w_router = np.asarray(inputs["w_router"], dtype=np.float32)
    Wg = np.asarray(inputs["Wg"], dtype=np.float32)
    Wu = np.asarray(inputs["Wu"], dtype=np.float32)
    Wd = np.asarray(inputs["Wd"], dtype=np.float32)
    sg = np.asarray(inputs["sg"], dtype=np.float32)
    su = np.asarray(inputs["su"], dtype=np.float32)
    sd = np.asarray(inputs["sd"], dtype=np.float32)

    xf = np.ascontiguousarray(x.reshape(_T, _D))
    i1, i2, w1, w2 = _route(xf, w_router)

    idxs, cws = [], []
    for e in range(_E):
        sel = (i1 == e) | (i2 == e)
        idx = np.nonzero(sel)[0]
        cwv = np.where(i1[idx] == e, w1[idx], w2[idx]).astype(np.float32)
        idxs.append(idx)
        cws.append(cwv)
    cmax = max(len(i) for i in idxs)
    C = max(512, int(math.ceil(cmax / 128.0)) * 128)

    xt_h = _tile_kxm(np.ascontiguousarray(xf.T), _DK)  # [P, DK, T]

    in_maps = []
    for e in range(_E):
        idx, cwv = idxs[e], cws[e]
        n = len(idx)
        xe_h = np.zeros((_P, _DK, C), np.float32)
        if n:
            xe_h[:, :, :n] = _tile_kxm(np.ascontiguousarray(xf[idx].T), _DK)
        cw_h = np.zeros((_P, C), np.float32)
        cw_h[:, :n] = cwv[None, :]

        wgT = np.ascontiguousarray(Wg[e].T)  # [D, H]
        wg_h = np.ascontiguousarray(
            wgT.reshape(_DK, _P, _HT, _P).transpose(2, 1, 0, 3)
        )
        wuT = np.ascontiguousarray(Wu[e].T)
        wu_h = np.ascontiguousarray(
            wuT.reshape(_DK, _P, _HT, _P).transpose(2, 1, 0, 3)
        )
        wdT = np.ascontiguousarray(Wd[e].T)  # [H, D]
        wd_h = np.ascontiguousarray(
            wdT.reshape(_HT, _P, _DK, _P).transpose(2, 1, 0, 3)
        )

        hs = slice(e * _HSH, (e + 1) * _HSH)
        sg_h = _tile_kxm(np.ascontiguousarray(sg[hs].T), _DK)  # [P, DK, HSH]
        su_h = _tile_kxm(np.ascontiguousarray(su[hs].T), _DK)
        sd_h = _tile_kxm(np.ascontiguousarray(sd[:, hs].T), _SK)  # [P, SK, D]

        in_maps.append(
            {
                "xe": xe_h,
                "cw": cw_h,
                "wg": wg_h,
                "wu": wu_h,
                "wd": wd_h,
                "xt": xt_h,
                "sg": sg_h,
                "su": su_h,
                "sd": sd_h,
            }
        )
    return in_maps, idxs, C


def _combine(results, idxs):
    out = np.zeros((_D, _T), np.float32)
    for e in range(_E):
        out += results[e]["shout"].reshape(_D, _T)
        idx = idxs[e]
        if len(idx):
            out[:, idx] += results[e]["rout"].reshape(_D, -1)[:, : len(idx)]
    return np.ascontiguousarray(out.T).reshape(_B, _S, _D).astype(np.float32)


def kernel(**inputs):
    from concourse import bass_utils

    in_maps, idxs, C = _prepare(inputs)
    nc = _build(C)
    res = bass_utils.run_bass_kernel_spmd(nc, in_maps, core_ids=list(range(_NC)))
    return _combine(res.results, idxs)


# revision 8
# speedup vs baseline: 1.2261x; 1.2261x over previous
"""Trainium2 Bass kernel for nn_MoEFFNBlock (B=2,S=2048,D=1024,H=2048,E=8,K=2).

Strategy (expert-parallel, 8 cores):
  host: fp32 router (softmax+top2, validated to match the jax reference
        selection), gather tokens per expert, fold the normalized top-k
        combine weight into the expert output on-device.
  core e: expert-e SwiGLU FFN over its <=C gathered tokens with fp32r
        matmuls (full PE rate, ~1e-4 rel err), plus a 256-wide H-shard of
        the shared expert over all T tokens.
  host: scatter-add per-expert outputs + sum the 8 shared-expert partials.

All matmul operands are laid out on host so every DMA is a large
contiguous transfer: weights pre-tiled to [tile, 128, kouter, 128].
"""

import json
import math

import numpy as np

_B, _S, _D, _H, _E = 2, 2048, 1024, 2048, 8
_T = _B * _S
_P = 128
_NC = 8
_HSH = _H // _NC  # shared-expert H columns per core
_DK = _D // _P  # 8 contraction tiles over D
_HT = _H // _P  # 16 tiles over H
_SK = _HSH // _P  # 2 contraction tiles over the H-shard
_TC = 512  # shared-expert token chunk

_TPB_ENGINES = {"PE", "Activation", "DVE", "Pool", "SP"}


def _split_waits(bir_bytes: bytes) -> bytes:
    """walrus in this container accepts only one sync-wait per TPB
    instruction; Tile's tail drain carries several. Hoist extras onto
    NoOps that run just before the instruction on the same engine."""
    m = json.loads(bir_bytes)
    ctr = 0
    for f in m["functions"]:
        blocks = f["blocks"]
        items = blocks.items() if isinstance(blocks, dict) else enumerate(blocks)
        for _bname, bb in items:
            new_insts = []
            for inst in bb["instructions"]:
                si = inst.get("sync_info") or {}
                ow = si.get("on_wait") or []
                if len(ow) > 1 and inst.get("engine") in _TPB_ENGINES:
                    for w in ow[:-1]:
                        ctr += 1
                        nop = {
                            "name": f"I-waitsplit-{ctr}",
                            "engine": inst["engine"],
                            "opcode": "NoOp",
                            "ins": [],
                            "outs": [],
                            "sync_info": {"on_wait": [w], "on_update": []},
                        }
                        if "debug" in inst:
                            nop["debug"] = inst["debug"]
                        new_insts.append(nop)
                    si["on_wait"] = [ow[-1]]
                new_insts.append(inst)
            bb["instructions"] = new_insts
    return json.dumps(m).encode()


def _chunks(C):
    """Column chunks of width 256..512 (fp32r needs >=256 cols for full PE
    rate). C must be a multiple of 128, C >= 512."""
    widths = []
    rem = C
    while rem >= 768:
        widths.append(512)
        rem -= 512
    if rem == 512:
        widths.append(512)
    elif rem >= 512:
        widths.append(rem - 256)
        widths.append(256)
    elif rem:
        widths.append(rem)
    ccs, o = [], 0
    for w in widths:
        assert 256 <= w <= 512
        ccs.append((o, w))
        o += w
    assert o == C
    return ccs


def _build(C):
    import concourse.bass as bass
    import concourse.mybir as mybir
    import concourse.tile as tile

    f32 = mybir.dt.float32
    f32r = mybir.dt.float32r
    Silu = mybir.ActivationFunctionType.Silu
    mult = mybir.AluOpType.mult

    nc = bass.Bass(trn_type="TRN2")
    xe = nc.dram_tensor("xe", [_P, _DK, C], f32r, kind="ExternalInput")
    cw = nc.dram_tensor("cw", [_P, C], f32, kind="ExternalInput")
    wg = nc.dram_tensor("wg", [_HT, _P, _DK, _P], f32r, kind="ExternalInput")
    wu = nc.dram_tensor("wu", [_HT, _P, _DK, _P], f32r, kind="ExternalInput")
    wd = nc.dram_tensor("wd", [_DK, _P, _HT, _P], f32r, kind="ExternalInput")
    xt = nc.dram_tensor("xt", [_P, _DK, _T], f32r, kind="ExternalInput")
    sg = nc.dram_tensor("sg", [_P, _DK, _HSH], f32r, kind="ExternalInput")
    su = nc.dram_tensor("su", [_P, _DK, _HSH], f32r, kind="ExternalInput")
    sd = nc.dram_tensor("sd", [_P, _SK, _D], f32r, kind="ExternalInput")
    rout = nc.dram_tensor("rout", [_DK, _P, C], f32, kind="ExternalOutput")
    shout = nc.dram_tensor("shout", [_DK, _P, _T], f32, kind="ExternalOutput")

    ccs = _chunks(C)

    with tile.TileContext(nc) as tc:
        with (
            tc.tile_pool(name="tmp", bufs=2) as tmp,
            tc.tile_pool(name="ps", bufs=2, space="PSUM") as psp,
            tc.tile_pool(name="bigS", bufs=1) as bigS,
            tc.tile_pool(name="cwg", bufs=1) as cwg,
            tc.tile_pool(name="strDW", bufs=2) as strDW,
        ):
            # PE warmup: dummy matmuls so HAM un-throttles while the
            # initial DMAs are in flight.
            wtile32 = cwg.tile([_P, 512], f32, name="wtile32")
            nc.vector.memset(wtile32[:], 0.0)
            wtile = cwg.tile([_P, 512], f32r, name="wtile")
            nc.vector.tensor_copy(wtile[:], wtile32[:])
            with tc.tile_pool(name="warmps", bufs=1, space="PSUM") as warmps:
                wps = warmps.tile([_P, 512], f32, name="wps")
                for i in range(60):
                    nc.tensor.matmul(
                        wps[:],
                        wtile[:, :_P],
                        wtile[:],
                        start=(i == 0),
                        stop=(i == 59),
                    )

            cw_sb = cwg.tile([_P, C], f32, name="cw_sb")
            g_sb = cwg.tile([_P, _HT, C], f32r, name="g_sb")
            sg_sb = bigS.tile([_P, _DK, _HSH], f32r, name="sg_sb")
            su_sb = bigS.tile([_P, _DK, _HSH], f32r, name="su_sb")
            sd_sb = bigS.tile([_P, _SK, _D], f32r, name="sd_sb")

            # ---------- phase R / h-stage: g = silu(Wg x) * (Wu x) * cw --
            with (
                tc.tile_pool(name="poolXE", bufs=1) as poolXE,
                tc.tile_pool(name="strGU", bufs=2) as strGU,
            ):
                # First-needed data first: xe chunk 0 and the first weight
                # tiles, then the rest, then phase-S weights.
                xe_sb = poolXE.tile([_P, _DK, C], f32r, name="xe_sb")
                c0_, cn_ = ccs[0]
                nc.sync.dma_start(
                    xe_sb[:, :, c0_ : c0_ + cn_], xe.ap()[:, :, c0_ : c0_ + cn_]
                )
                wgu_tiles = []
                for ht in range(2):
                    wg_t = strGU.tile([_P, _DK, _P], f32r, tag="wg", name="wg_t")
                    nc.sync.dma_start(wg_t[:], wg.ap()[ht])
                    wu_t = strGU.tile([_P, _DK, _P], f32r, tag="wu", name="wu_t")
                    nc.sync.dma_start(wu_t[:], wu.ap()[ht])
                    wgu_tiles.append((wg_t, wu_t))
                for c0_, cn_ in ccs[1:]:
                    nc.sync.dma_start(
                        xe_sb[:, :, c0_ : c0_ + cn_], xe.ap()[:, :, c0_ : c0_ + cn_]
                    )
                nc.sync.dma_start(cw_sb[:], cw.ap())
                nc.sync.dma_start(sg_sb[:], sg.ap())
                nc.sync.dma_start(su_sb[:], su.ap())
                nc.sync.dma_start(sd_sb[:], sd.ap())

                for ht in range(_HT):
                    if ht < 2:
                        wg_t, wu_t = wgu_tiles[ht]
                    else:
                        wg_t = strGU.tile([_P, _DK, _P], f32r, tag="wg", name="wg_t")
                        nc.sync.dma_start(wg_t[:], wg.ap()[ht])
                        wu_t = strGU.tile([_P, _DK, _P], f32r, tag="wu", name="wu_t")
                        nc.sync.dma_start(wu_t[:], wu.ap()[ht])
                    for c0, cn in ccs:
                        h1 = psp.tile([_P, 512], f32, tag="h1", name="h1ps")[:, :cn]
                        for k in range(_DK):
                            nc.tensor.matmul(
                                h1,
                                wg_t[:, k],
                                xe_sb[:, k, c0 : c0 + cn],
                                start=(k == 0),
                                stop=(k == _DK - 1),
                            )
                        h2 = psp.tile([_P, 512], f32, tag="h2", name="h2ps", bufs=1)[:, :cn]
                        for k in range(_DK):
                            nc.tensor.matmul(
                                h2,
                                wu_t[:, k],
                                xe_sb[:, k, c0 : c0 + cn],
                                start=(k == 0),
                                stop=(k == _DK - 1),
                            )
                        sl = tmp.tile([_P, 512], f32, tag="sl", name="sl_sb")[
                            :, :cn
                        ]
                        nc.scalar.activation(sl, h1, Silu)
                        t2 = tmp.tile([_P, 512], f32, tag="t2", name="t2_sb")[
                            :, :cn
                        ]
                        nc.vector.tensor_tensor(t2, h2, cw_sb[:, c0 : c0 + cn], mult)
                        nc.vector.tensor_tensor(
                            g_sb[:, ht, c0 : c0 + cn], sl, t2, mult
                        )

            # ---------- phase R / d-stage + phase S ----------------------
            with (
                tc.tile_pool(name="strS", bufs=3) as strS,
            ):
                for dt_i in range(_DK):
                    wd_t = strDW.tile([_P, _HT, _P], f32r, tag="wd", name="wd_t")
                    nc.sync.dma_start(wd_t[:], wd.ap()[dt_i])
                    for c0, cn in ccs:
                        ops = psp.tile(
                            [_P, 512], f32, tag="out", name="ops", bufs=4
                        )[:, :cn]
                        for k in range(_HT):
                            nc.tensor.matmul(
                                ops,
                                wd_t[:, k],
                                g_sb[:, k, c0 : c0 + cn],
                                start=(k == 0),
                                stop=(k == _HT - 1),
                            )
                        ro = tmp.tile([_P, 512], f32, tag="ro", name="ro_sb", bufs=6)[
                            :, :cn
                        ]
                        if dt_i % 2:
                            nc.scalar.copy(ro, ops)
                        else:
                            nc.vector.tensor_copy(ro, ops)
                        nc.sync.dma_start(rout.ap()[dt_i][:, c0 : c0 + cn], ro)

                # phase S, software-pipelined: d-stage trails one chunk.
                def s_hstage(t0):
                    xt_sb = strS.tile([_P, _DK, _TC], f32r, tag="xt", name="xt_sb")
                    nc.sync.dma_start(xt_sb[:], xt.ap()[:, :, t0 : t0 + _TC])
                    gs = strS.tile([_P, _SK, _TC], f32r, tag="gs", name="gs_sb")
                    for hs in range(_SK):
                        h1 = psp.tile([_P, 512], f32, tag="h1", name="h1ps")
                        for k in range(_DK):
                            nc.tensor.matmul(
                                h1,
                                sg_sb[:, k, hs * _P : (hs + 1) * _P],
                                xt_sb[:, k],
                                start=(k == 0),
                                stop=(k == _DK - 1),
                            )
                        h2 = psp.tile([_P, 512], f32, tag="h2", name="h2ps", bufs=1)
                        for k in range(_DK):
                            nc.tensor.matmul(
                                h2,
                                su_sb[:, k, hs * _P : (hs + 1) * _P],
                                xt_sb[:, k],
                                start=(k == 0),
                                stop=(k == _DK - 1),
                            )
                        sl = tmp.tile([_P, 512], f32, tag="sl", name="sl_sb")
                        nc.scalar.activation(sl, h1, Silu)
                        nc.vector.tensor_tensor(gs[:, hs], sl, h2, mult)
                    return gs

                def s_dstage(t0, gs):
                    for dt_i in range(_DK):
                        ops = psp.tile(
                            [_P, 512], f32, tag="out", name="ops", bufs=4
                        )
                        for k in range(_SK):
                            nc.tensor.matmul(
                                ops,
                                sd_sb[:, k, dt_i * _P : (dt_i + 1) * _P],
                                gs[:, k],
                                start=(k == 0),
                                stop=(k == _SK - 1),
                            )
                        so = tmp.tile([_P, 512], f32, tag="ro", name="ro_sb", bufs=6)
                        if dt_i % 2:
                            nc.scalar.copy(so, ops)
                        else:
                            nc.vector.tensor_copy(so, ops)
                        nc.sync.dma_start(shout.ap()[dt_i][:, t0 : t0 + _TC], so)

                prev = None
                for t0 in range(0, _T, _TC):
                    gs = s_hstage(t0)
                    if prev is not None:
                        s_dstage(prev[0], prev[1])
                    prev = (t0, gs)
                s_dstage(prev[0], prev[1])

    orig = nc.to_json_bytes
    nc.to_json_bytes = lambda: _split_waits(orig())
    return nc


def _route(xf, w_router):
    """fp32 router matching the jax reference: softmax over logits, top-2
    (selection identical to jax.lax.top_k for non-tied logits), weights
    renormalized over the selected pair."""
    logits = xf @ w_router.T.astype(np.float32)
    m = logits.max(-1, keepdims=True)
    p = np.exp(logits - m)
    p /= p.sum(-1, keepdims=True)
    i1 = p.argmax(-1)
    p2 = p.copy()
    p2[np.arange(p.shape[0]), i1] = -1.0
    i2 = p2.argmax(-1)
    w1 = p[np.arange(p.shape[0]), i1]
    w2 = p[np.arange(p.shape[0]), i2]
    s = w1 + w2
    return i1, i2, (w1 / s).astype(np.float32), (w2 / s).astype(np.float32)


def _tile_kxm(a2d, kouter):
    """[K, M] -> [128, K//128, M] with partition dim first."""
    k, mdim = a2d.shape
    assert k == kouter * _P
    return np.ascontiguousarray(a2d.reshape(kouter, _P, mdim).transpose(1, 0, 2))


def _prepare(inputs):
    x = np.asarray(inputs["x"], dtype=np.float32)
    w_router = np.asarray(inputs["w_router"], dtype=np.float32)
    Wg = np.asarray(inputs["Wg"], dtype=np.float32)
    Wu = np.asarray(inputs["Wu"], dtype=np.float32)
    Wd = np.asarray(inputs["Wd"], dtype=np.float32)
    sg = np.asarray(inputs["sg"], dtype=np.float32)
    su = np.asarray(inputs["su"], dtype=np.float32)
    sd = np.asarray(inputs["sd"], dtype=np.float32)

    xf = np.ascontiguousarray(x.reshape(_T, _D))
    i1, i2, w1, w2 = _route(xf, w_router)

    idxs, cws = [], []
    for e in range(_E):
        sel = (i1 == e) | (i2 == e)
        idx = np.nonzero(sel)[0]
        cwv = np.where(i1[idx] == e, w1[idx], w2[idx]).astype(np.float32)
        idxs.append(idx)
        cws.append(cwv)
    cmax = max(len(i) for i in idxs)
    C = max(512, int(math.ceil(cmax / 128.0)) * 128)

    xt_h = _tile_kxm(np.ascontiguousarray(xf.T), _DK)  # [P, DK, T]

    in_maps = []
    for e in range(_E):
        idx, cwv = idxs[e], cws[e]
        n = len(idx)
        xe_h = np.zeros((_P, _DK, C), np.float32)
        if n:
            xe_h[:, :, :n] = _tile_kxm(np.ascontiguousarray(xf[idx].T), _DK)
        cw_h = np.zeros((_P, C), np.float32)
        cw_h[:, :n] = cwv[None, :]

        wgT = np.ascontiguousarray(Wg[e].T)  # [D, H]
        wg_h = np.ascontiguousarray(
            wgT.reshape(_DK, _P, _HT, _P).transpose(2, 1, 0, 3)
        )
        wuT = np.ascontiguousarray(Wu[e].T)
        wu_h = np.ascontiguousarray(
            wuT.reshape(_DK, _P, _HT, _P).transpose(2, 1, 0, 3)
        )
        wdT = np.ascontiguousarray(Wd[e].T)  # [H, D]
        wd_h = np.ascontiguousarray(
            wdT.reshape(_HT, _P, _DK, _P).transpose(2, 1, 0, 3)
        )

        hs = slice(e * _HSH, (e + 1) * _HSH)
        sg_h = _tile_kxm(np.ascontiguousarray(sg[hs].T), _DK)  # [P, DK, HSH]
        su_h = _tile_kxm(np.ascontiguousarray(su[hs].T), _DK)
        sd_h = _tile_kxm(np.ascontiguousarray(sd[:, hs].T), _SK)  # [P, SK, D]

        in_maps.append(
            {
                "xe": xe_h,
                "cw": cw_h,
                "wg": wg_h,
                "wu": wu_h,
                "wd": wd_h,
                "xt": xt_h,
                "sg": sg_h,
                "su": su_h,
                "sd": sd_h,
            }
        )
    return in_maps, idxs, C


def _combine(results, idxs):
    out = np.zeros((_D, _T), np.float32)
    for e in range(_E):
        out += results[e]["shout"].reshape(_D, _T)
        idx = idxs[e]
        if len(idx):
            out[:, idx] += results[e]["rout"].reshape(_D, -1)[:, : len(idx)]
    return np.ascontiguousarray(out.T).reshape(_B, _S, _D).astype(np.float32)


def kernel(**inputs):
    from concourse import bass_utils

    in_maps, idxs, C = _prepare(inputs)
    nc = _build(C)
    res = bass_utils.run_bass_kernel_spmd(nc, in_maps, core_ids=list(range(_NC)))
    return _combine(res.results, idxs)
